# revision 15
# baseline (speedup 1.0000x reference)
"""MoE layer (12 experts, top-3, SwiGLU) Trainium2 Bass kernel.

Data-parallel over tokens: 16384 tokens sharded as 2048/core across 8 cores,
expert weights replicated. Routing (gate matmul, softmax-free top-3, combine
weights) in fp32 on device; expert matmuls in fp16 (1 cycle/row on PE, ~2^-11
relative precision); final combine accumulated in fp32.

Single pool scope so the Tile scheduler overlaps routing, expert compute and
DMA; weight/activation DMAs are split so the first expert matmul starts after
a fraction of the input traffic.

Outputs mirror reference:  (final, router_logits, selected_experts, router_loss)
"""

import os
import sys

for _p in ("/opt/trn_rl_repo",):
    if _p not in sys.path and os.path.isdir(_p):
        sys.path.insert(0, _p)

import numpy as np

import concourse.bass as bass
import concourse.mybir as mybir
from concourse import bacc
from concourse.tile import TileContext

# ---- problem constants (hardcoded; kernel.py must be self-contained) ----
N_TOKENS = 16384
HIDDEN = 1024
FFN = 256
N_EXPERTS = 12
TOP_K = 3
N_CORES = 8

P = 128
TOK_CORE_FULL = N_TOKENS // N_CORES  # 2048
KT = HIDDEN // P                     # 8 k-tiles over hidden
FC = FFN // P                        # 2 f-chunks over FFN

f32 = mybir.dt.float32
f16 = mybir.dt.float16
u32 = mybir.dt.uint32
Alu = mybir.AluOpType
Act = mybir.ActivationFunctionType
AX = mybir.AxisListType


def build_bass(tok_core: int = TOK_CORE_FULL, use_silu: bool = True) -> bass.Bass:
    """Build the per-core Bass program.

    Per-core inputs:
      xt   [HIDDEN, tok_core] f32    (transposed hidden slice, for the gate)
      xt16 [HIDDEN, tok_core] f16    (transposed hidden slice, for experts)
      w13  [HIDDEN, N_EXPERTS*2*FFN] f16  ([d][e, mat(w1/w3), f] layout)
      w2   [FFN, N_EXPERTS*HIDDEN] f16    ([f][e, d] layout)
      gate [HIDDEN, N_EXPERTS] f32        (gate_w.T)
    Per-core outputs:
      final  [tok_core, HIDDEN] f32
      logits [tok_core, N_EXPERTS] f32
      sel    [tok_core, TOP_K] u32
      imp    [N_EXPERTS, 1] f32  (partial importance sums)
    """
    NT = tok_core // P        # 128-token tiles
    TB = 4 * P                # expert-phase token block
    NEB = tok_core // TB
    assert tok_core % TB == 0

    nc = bacc.Bacc(
        "TRN2",
        target_bir_lowering=False,
        debug=False,
        num_devices=N_CORES,
    )

    xt_d = nc.declare_dram_parameter("xt", [HIDDEN, tok_core], f32, isOutput=False)
    xt16_d = nc.declare_dram_parameter("xt16", [HIDDEN, tok_core], f16, isOutput=False)
    w13_d = nc.declare_dram_parameter(
        "w13", [HIDDEN, N_EXPERTS * 2 * FFN], f16, isOutput=False
    )
    w2_d = nc.declare_dram_parameter(
        "w2", [FFN, N_EXPERTS * HIDDEN], f16, isOutput=False
    )
    gate_d = nc.declare_dram_parameter(
        "gate", [HIDDEN, N_EXPERTS], f32, isOutput=False
    )

    final_d = nc.declare_dram_parameter("final", [tok_core, HIDDEN], f32, isOutput=True)
    logits_d = nc.declare_dram_parameter(
        "logits", [tok_core, N_EXPERTS], f32, isOutput=True
    )
    sel_d = nc.declare_dram_parameter("sel", [tok_core, TOP_K], u32, isOutput=True)
    imp_d = nc.declare_dram_parameter("imp", [N_EXPERTS, 1], f32, isOutput=True)

    xt_v = xt_d[:].rearrange("(kt p) t -> p kt t", p=P)
    xt16_v = xt16_d[:].rearrange("(kt p) t -> p kt t", p=P)
    w13_v = w13_d[:].rearrange("(kt p) m -> p kt m", p=P)
    w2_v = w2_d[:].rearrange("(fc p) m -> p fc m", p=P)
    gate_v = gate_d[:].rearrange("(kt p) e -> p kt e", p=P)

    with TileContext(nc) as tc:
        with (
            tc.tile_pool(name="wpool", bufs=1) as wpool,
            tc.tile_pool(name="pro", bufs=2) as pro,
            tc.tile_pool(name="es", bufs=2) as es,
            tc.tile_pool(name="hpsum", bufs=2, space="PSUM") as hpsum,
            tc.tile_pool(name="opsum", bufs=3, space="PSUM") as opsum,
            tc.tile_pool(name="rpsum", bufs=1, space="PSUM") as rpsum,
        ):
            gate_sb = wpool.tile([P, KT, N_EXPERTS], f32)
            nc.sync.dma_start(gate_sb[:], gate_v)
            ones_sb = wpool.tile([P, 1], f32)
            nc.vector.memset(ones_sb[:], 1.0)
            w13_sb = wpool.tile([P, KT, N_EXPERTS * 2 * FFN], f16)
            w2_sb = wpool.tile([P, FC, N_EXPERTS * HIDDEN], f16)
            xbf = wpool.tile([P, KT, tok_core], f16)
            comb = wpool.tile([P, NT, N_EXPERTS], f32)
            imp_sb = wpool.tile([N_EXPERTS, NT], f32)

            def dma_w(e):
                es_ = slice(e * 2 * FFN, (e + 1) * 2 * FFN)
                nc.sync.dma_start(w13_sb[:, :, es_], w13_v[:, :, es_])
                eh = slice(e * HIDDEN, (e + 1) * HIDDEN)
                nc.sync.dma_start(w2_sb[:, :, eh], w2_v[:, :, eh])

            def dma_x(b):
                bs = slice(b * TB, (b + 1) * TB)
                nc.sync.dma_start(xbf[:, :, bs], xt16_v[:, :, bs])

            # issue order = consumption order: expert-0 weights and block-0
            # activations first so mm1 starts ~7us in; the rest streams behind
            # the routing loop's xf tiles.
            dma_w(0)
            dma_x(0)

            # ---------------- routing (overlaps expert phase) ----------------
            for t in range(NT):
                ts = slice(t * P, (t + 1) * P)
                xf = pro.tile([P, KT, P], f32, tag="xf", bufs=1)
                nc.sync.dma_start(xf[:], xt_v[:, :, ts])
                # gate logits (fp32 matmul); psum shares the mm2 pool slots
                lg_ps = rpsum.tile([P, N_EXPERTS], f32, tag="r", name="lg_ps")
                for kt in range(KT):
                    nc.tensor.matmul(
                        lg_ps[:],
                        lhsT=xf[:, kt, :],
                        rhs=gate_sb[:, kt, :],
                        start=(kt == 0),
                        stop=(kt == KT - 1),
                    )
                logits_sb = pro.tile([P, N_EXPERTS], f32, tag="logits")
                nc.scalar.copy(logits_sb[:], lg_ps[:])
                nc.sync.dma_start(logits_d[ts, :], logits_sb[:])
                # exp(logits); top-k of softmax == top-k of exp(logits)
                ew = pro.tile([P, N_EXPERTS], f32, tag="ew")
                nc.scalar.activation(ew[:], lg_ps[:], Act.Exp)
                top8 = pro.tile([P, 8], f32, tag="top8")
                nc.vector.max(out=top8[:], in_=ew[:])
                idx8 = pro.tile([P, 8], u32, tag="idx8")
                nc.vector.max_index(out=idx8[:], in_max=top8[:], in_values=ew[:])
                nc.sync.dma_start(sel_d[ts, :], idx8[:, 0:TOP_K])
                # renormalized top-3 weights scattered into comb[t]
                s3 = pro.tile([P, 1], f32, tag="s3")
                nc.vector.reduce_sum(s3[:], top8[:, 0:TOP_K], axis=AX.X)
                r3 = pro.tile([P, 1], f32, tag="r3")
                nc.vector.reciprocal(r3[:], s3[:])
                nc.vector.memset(top8[:, TOP_K:8], -1.0)
                zap = pro.tile([P, N_EXPERTS], f32, tag="zap")
                nc.vector.match_replace(
                    out=zap[:], in_to_replace=top8[:], in_values=ew[:],
                    imm_value=0.0,
                )
                kept = pro.tile([P, N_EXPERTS], f32, tag="kept")
                nc.vector.tensor_sub(kept[:], ew[:], zap[:])
                nc.vector.tensor_scalar_mul(comb[:, t, :], kept[:], r3[:])
                # importance column-sums via PE; accumulate in SBUF per tile
                imp_ps = rpsum.tile([N_EXPERTS, 1], f32, tag="r", name="imp_ps")
                nc.tensor.matmul(
                    imp_ps[:], lhsT=comb[:, t, :], rhs=ones_sb[:],
                    start=True, stop=True,
                )
                nc.scalar.copy(imp_sb[:, t : t + 1], imp_ps[:])
            imp_tot = pro.tile([N_EXPERTS, 1], f32, tag="imp_tot", bufs=1)
            nc.vector.reduce_sum(imp_tot[:], imp_sb[:], axis=AX.X)
            nc.sync.dma_start(imp_d[:], imp_tot[:])

            # remaining weight/activation streams (behind the routing xf DMAs)
            for e in range(1, N_EXPERTS):
                dma_w(e)
            for b in range(1, NEB):
                dma_x(b)

            # ---------------- experts ----------------
            # 512-token blocks; mm1 streams N=512. Per expert: for each
            # f-chunk, h1|h3 into a [128, 2, 512] psum tile, SwiGLU into fp16
            # hw, then per-128-token mm2 + weighted accumulate into fp32.
            for b in range(NEB):
                bs = slice(b * TB, (b + 1) * TB)
                fins = [
                    es.tile(
                        [P, 2, HIDDEN], f32, tag=f"fin{half}", name=f"fin{half}",
                        bufs=1,
                    )
                    for half in range(2)
                ]
                for e in range(N_EXPERTS):
                    hw = es.tile([P, FC, TB], f16, tag="hw")
                    for fc in range(FC):
                        h_ps = hpsum.tile([P, 2, TB], f32, tag="h")
                        for mat in range(2):
                            off = (e * 2 + mat) * FFN + fc * P
                            for kt in range(KT):
                                nc.tensor.matmul(
                                    h_ps[:, mat, :],
                                    lhsT=w13_sb[:, kt, off : off + P],
                                    rhs=xbf[:, kt, bs],
                                    start=(kt == 0),
                                    stop=(kt == KT - 1),
                                )
                        sig = es.tile([P, TB], f32, tag="sig", bufs=1)
                        if use_silu:
                            # silu(h1) directly on ACT (jax.nn.silu = x*sigmoid(x))
                            nc.scalar.activation(sig[:], h_ps[:, 0, :], Act.Silu)
                        else:
                            # CoreSim fallback: sigmoid then in-place multiply
                            nc.scalar.activation(sig[:], h_ps[:, 0, :], Act.Sigmoid)
                            nc.vector.tensor_mul(sig[:], sig[:], h_ps[:, 0, :])
                        nc.vector.tensor_mul(hw[:, fc, :], sig[:], h_ps[:, 1, :])
                    for th in range(4):
                        wsl = comb[:, b * 4 + th, e : e + 1]
                        fin = fins[th // 2]
                        for nch in range(2):
                            o_ps = opsum.tile([P, 512], f32, tag="o", name="o_ps")
                            for fc in range(FC):
                                nc.tensor.matmul(
                                    o_ps[:],
                                    lhsT=hw[:, fc, th * P : (th + 1) * P],
                                    rhs=w2_sb[
                                        :, fc,
                                        e * HIDDEN + nch * 512 : e * HIDDEN + (nch + 1) * 512,
                                    ],
                                    start=(fc == 0),
                                    stop=(fc == FC - 1),
                                )
                            dsl = fin[:, th % 2, nch * 512 : (nch + 1) * 512]
                            if e == 0:
                                nc.vector.tensor_scalar_mul(dsl, o_ps[:], wsl)
                            else:
                                nc.vector.scalar_tensor_tensor(
                                    out=dsl,
                                    in0=o_ps[:],
                                    scalar=wsl,
                                    in1=dsl,
                                    op0=Alu.mult,
                                    op1=Alu.add,
                                )
                for half in range(2):
                    hs = slice(b * TB + half * 2 * P, b * TB + (half + 1) * 2 * P)
                    nc.sync.dma_start(
                        final_d[hs, :].rearrange("(th p) d -> p th d", p=P),
                        fins[half][:],
                    )

    nc.finalize()
    return nc


# ---- host-side packing (shared by all cores) ----

def _pack_weights(gate_w, w1, w2, w3):
    # w13 [HIDDEN, e, mat, f] fp16
    w13 = np.stack([w1, w3], axis=1)  # [e, 2, f, d]
    w13 = np.ascontiguousarray(w13.transpose(3, 0, 1, 2)).reshape(
        HIDDEN, N_EXPERTS * 2 * FFN
    )
    w13 = w13.astype(np.float16)
    # w2t [FFN, e, d] fp16
    w2t = np.ascontiguousarray(w2.transpose(2, 0, 1)).reshape(
        FFN, N_EXPERTS * HIDDEN
    ).astype(np.float16)
    gt = np.ascontiguousarray(gate_w.T.astype(np.float32))  # [HIDDEN, 12]
    return w13, w2t, gt


_CACHE: dict = {}


def kernel(hidden_states, gate_w, w1, w2, w3):
    hidden_states = np.asarray(hidden_states, dtype=np.float32)
    gate_w = np.asarray(gate_w, dtype=np.float32)
    w1 = np.asarray(w1, dtype=np.float32)
    w2 = np.asarray(w2, dtype=np.float32)
    w3 = np.asarray(w3, dtype=np.float32)

    from concourse.bass_utils import run_bass_kernel_spmd

    if "nc" not in _CACHE:
        _CACHE["nc"] = build_bass()
    nc = _CACHE["nc"]

    w13, w2t, gt = _pack_weights(gate_w, w1, w2, w3)

    in_maps = []
    for c in range(N_CORES):
        xs = hidden_states[c * TOK_CORE_FULL : (c + 1) * TOK_CORE_FULL]
        xt = np.ascontiguousarray(xs.T)  # [HIDDEN, 2048]
        in_maps.append(
            {
                "xt": xt,
                "xt16": xt.astype(np.float16),
                "w13": w13,
                "w2": w2t,
                "gate": gt,
            }
        )

    res = run_bass_kernel_spmd(
        nc,
        in_maps,
        list(range(N_CORES)),
        trace=bool(int(os.environ.get("KERNEL_TRACE", "0"))),
    )
    _CACHE["last_exec_time_ns"] = res.exec_time_ns

    final = np.concatenate([res.results[c]["final"] for c in range(N_CORES)], axis=0)
    logits = np.concatenate([res.results[c]["logits"] for c in range(N_CORES)], axis=0)
    sel = np.concatenate(
        [res.results[c]["sel"].astype(np.int32) for c in range(N_CORES)], axis=0
    )
    imp = np.sum(
        np.stack([res.results[c]["imp"][:, 0] for c in range(N_CORES)]), axis=0
    ).astype(np.float32)
    # torch.std / jnp.std with ddof=1
    router_loss = np.float32(np.std(imp, ddof=1))

    return final, logits, sel, router_loss
